# revision 51
# baseline (speedup 1.0000x reference)
"""GAT (2-layer, 8-head) Trainium2 Bass kernel, 8-core row-parallel SPMD.

Sharding: nodes (rows of x / adj) split across 8 cores, 512 rows each. Each core
computes its rows' attention against all 4096 nodes; small weights replicated.
Attention math runs transposed (source j on partitions, my row i on free dim)
so aggregation attn @ Wh maps onto the PE with no transposition of the big
attention matrix.

Key algebra (per head): with z = src_i + dst_j,
  p = adj * exp(leaky_relu(z)) = adj * e^{0.2 z} * max(e^{0.8 z}, 1)
e^{0.2 s_i} factors out of numerator and denominator of the softmax and
cancels, so per (j-chunk, i) only two elementwise ops remain:
  M08 = max(embs * e02d_j, e1dm_j)      (DVE tensor_scalar, bf16 -> 4x mode)
  PT  = M08 * adjT_j                    (mask multiply; adjT is {0,1} bf16)
  outT[d,i] += [Wh | 1][j,d] * PT[j,i]  (PE matmul bf16, row 64 = denom)
adj is pre-transposed and cast to bf16 {0,1e30} on the host, so no on-device
transposes of the big matrices are needed. The two hot ops per chunk are
distributed over DVE / GPSIMD / ScalarE by a tunable pattern.

Cross-core communication: AllGather of layer-2 Wh2.T (1 MB) and one AllReduce
carrying [sum(x), sum(x^2)] for one-pass batchnorm statistics.
"""

import numpy as np
from contextlib import ExitStack

import concourse.bass as bass
import concourse.bacc as bacc
import concourse.tile as tile
from concourse import mybir
from concourse.bass_utils import run_bass_kernel_spmd

F32 = mybir.dt.float32
F32R = mybir.dt.float32r
BF16 = mybir.dt.bfloat16
I32 = mybir.dt.int32
AF = mybir.ActivationFunctionType
ALU = mybir.AluOpType

N_CORES = 8
N = 4096
NIN = 128
NHID = 64
NOUT = 64
H = 8
MY = N // N_CORES          # 512 rows per core
NJC = N // 128             # 32 j-chunks
NIB = MY // 128            # 4 row blocks per core
ALPHA = 0.2
EPS = 1e-5

CFG = "bf16"

# Per-chunk engine assignment for the two hot elementwise ops, cycled over
# j-chunks. 1: DVE TS + DVE TT(min)   2: DVE TS + Pool TT(min)
# 3: Act mul + DVE STT(max,min) -- strictly worse than 1, unused.
# Balanced so DVE / Pool busy times roughly equalize.
PATTERN = [2, 1, 1, 2, 1, 1, 2, 1, 1, 2, 1, 1, 2, 1, 1, 2,
           1, 1, 2, 1, 1, 2, 1, 1, 2, 1, 1, 2, 1, 2, 1, 1]

_CACHED = {}


def build_program(sim=False, cfg=None):
    nd = 1 if sim else N_CORES
    nc = bacc.Bacc("TRN2", target_bir_lowering=False, debug=False, num_devices=nd)

    d = {}
    d["xtf"] = nc.dram_tensor("xtf", [NIN, N], F32, kind="ExternalInput")
    d["xmtf"] = nc.dram_tensor("xmtf", [NIN, MY], F32, kind="ExternalInput")
    d["adjtb"] = nc.dram_tensor("adjtb", [128, NJC, MY], BF16,
                                kind="ExternalInput")
    d["wallf"] = nc.dram_tensor("wallf", [NIN, H * NHID], F32,
                                kind="ExternalInput")
    d["xtb"] = nc.dram_tensor("xtb", [NIN, N], BF16, kind="ExternalInput")
    d["wallb"] = nc.dram_tensor("wallb", [NIN, H * NHID], BF16,
                                kind="ExternalInput")
    d["vall"] = nc.dram_tensor("vall", [NIN, 2 * H], F32, kind="ExternalInput")
    d["sel8"] = nc.dram_tensor("sel8", [H, H, 128], F32, kind="ExternalInput")
    d["woutb"] = nc.dram_tensor("woutb", [128, 4, NOUT], F32,
                                kind="ExternalInput")
    d["a2"] = nc.dram_tensor("a2", [NOUT, 2], F32, kind="ExternalInput")
    d["gb"] = nc.dram_tensor("gb", [NOUT, 2], F32, kind="ExternalInput")
    d["ident"] = nc.dram_tensor("ident", [128, 128], F32, kind="ExternalInput")
    d["y"] = nc.dram_tensor("y", [MY, NOUT], F32, kind="ExternalOutput")

    with tile.TileContext(nc) as tc:
        with ExitStack() as ctx:
            _build_body(nc, tc, ctx, d, sim=sim)
    nc.compile()
    return nc


def _build_body(nc, tc, ctx, d, sim):
    def collective(kind, op, in_tile, out_tile, src_sbuf=None):
        if sim:
            if kind == "AllGather":
                nc.sync.dma_start(out=out_tile[0], in_=in_tile.opt())
            elif src_sbuf is not None:
                nc.sync.dma_start(out=out_tile.opt(), in_=src_sbuf)
            else:
                nc.sync.dma_start(out=out_tile.opt(), in_=in_tile.opt())
        else:
            nc.gpsimd.collective_compute(
                kind, op, replica_groups=[list(range(N_CORES))],
                ins=[in_tile.opt()], outs=[out_tile.opt()])

    consts = ctx.enter_context(tc.tile_pool(name="consts", bufs=1))
    persist = ctx.enter_context(tc.tile_pool(name="persist", bufs=1))
    dram = ctx.enter_context(tc.tile_pool(name="dram", bufs=1, space="DRAM"))
    hot = ctx.enter_context(tc.tile_pool(name="hot", bufs=10))
    agg = ctx.enter_context(tc.tile_pool(name="agg", bufs=3, space="PSUM"))
    post = ctx.enter_context(tc.tile_pool(name="post", bufs=2))
    postp = ctx.enter_context(tc.tile_pool(name="postp", bufs=2, space="PSUM"))
    ctx_a = ExitStack()

    # ---------------- inputs to SBUF ----------------
    # Critical phase-A inputs dispatch first, spread over the two HWDGE
    # queues (SP / Act) to limit serial descriptor-generation delay.
    adjtb = persist.tile([128, NJC, MY], BF16)   # {0,1}; adj[i, 128*jc+jp]

    def adjtb_dma(q, eng):
        eng.dma_start(out=adjtb[:, 4 * q:4 * (q + 1), :],
                      in_=d["adjtb"].ap()[:, 4 * q:4 * (q + 1), :])

    inp = ctx_a.enter_context(tc.tile_pool(name="inp", bufs=1))
    xtf = inp.tile([128, N], F32)
    nc.sync.dma_start(out=xtf, in_=d["xtf"].ap())
    adjtb_dma(0, nc.scalar)
    vall = inp.tile([128, 2 * H], F32)
    nc.scalar.dma_start(out=vall, in_=d["vall"].ap())
    xmtf = inp.tile([128, MY], F32)
    nc.scalar.dma_start(out=xmtf, in_=d["xmtf"].ap())
    adjtb_dma(1, nc.sync)
    sel8 = inp.tile([H, H, 128], F32)
    nc.scalar.dma_start(out=sel8, in_=d["sel8"].ap())
    wallb = inp.tile([128, H * NHID], BF16)
    nc.scalar.dma_start(out=wallb, in_=d["wallb"].ap())
    xtb = inp.tile([128, N], BF16)
    nc.sync.dma_start(out=xtb, in_=d["xtb"].ap())
    ident = consts.tile([128, 128], F32)
    nc.sync.dma_start(out=ident, in_=d["ident"].ap())
    for q, eng in ((2, nc.scalar), (3, nc.sync), (4, nc.scalar),
                   (5, nc.scalar), (6, nc.sync), (7, nc.sync)):
        adjtb_dma(q, eng)

    ones_row = consts.tile([1, 128], F32)
    nc.gpsimd.memset(ones_row, 1.0)
    negones_row = consts.tile([1, 128], F32)
    nc.gpsimd.memset(negones_row, -1.0)

    woutb = consts.tile([128, 4, NOUT], F32)
    nc.sync.dma_start(out=woutb, in_=d["woutb"].ap())
    a2 = consts.tile([NOUT, 2], F32)
    nc.scalar.dma_start(out=a2, in_=d["a2"].ap())
    gb = consts.tile([NOUT, 2], F32)
    nc.sync.dma_start(out=gb, in_=d["gb"].ap())

        # ---------------- persistent intermediates ----------------
    dstc = persist.tile([128, NJC, H], F32)     # per-head dst[128*jc+jp]
    e1dmc = persist.tile([128, NJC, H], F32)    # exp(dst - m_h)
    e02dc = persist.tile([128, NJC, H], F32)    # exp(0.2*dst)
    embsb = persist.tile([128, H, MY], BF16)    # exp(-0.8*src - m_h) bcast
    negmb = persist.tile([128, H], F32)         # -m_h (per-head dst max)
    whcall = persist.tile([128, NJC, H, NHID + 1], BF16)  # [Wh | 1] lhsT
    hcatT = persist.tile([128, NIB, MY], F32)   # layer-1 output (transposed)


    # ============ PHASE A: x-side precompute ==============
    with ctx_a as actx:
        pa = actx.enter_context(tc.tile_pool(name="pa", bufs=4))
        whp = actx.enter_context(tc.tile_pool(name="whp", bufs=3, space="PSUM"))

        # dst[j,h] = x[j,:] @ (W_h @ a_dst_h) with host-precomputed v
        dstps = whp.tile([128, NJC, H], F32, tag="ps")
        for jc in range(NJC):
            nc.tensor.matmul(dstps[:, jc, :], xtf[:, 128 * jc:128 * (jc + 1)],
                             vall[:, 0:H], start=True, stop=True)
        nc.scalar.copy(dstc, dstps)
        nc.scalar.activation(e02dc, dstc, AF.Exp, scale=0.2)
        # per-head m_h = max_j dst_h[j]; negmb[:, h] = -m_h broadcast
        mxp = pa.tile([128, H], F32, tag="mxp")
        for h in range(H):
            nc.vector.tensor_reduce(mxp[:, h:h + 1], dstc[:, :, h],
                                    axis=mybir.AxisListType.X, op=ALU.max)
        mxrow = pa.tile([1, H], F32, tag="mxrow")
        nc.gpsimd.tensor_reduce(mxrow, mxp, axis=mybir.AxisListType.C,
                                op=ALU.max)
        psm = whp.tile([128, 512], F32, tag="ps")
        nc.tensor.matmul(psm[:, 0:H], negones_row, mxrow,
                         start=True, stop=True)
        nc.vector.tensor_copy(negmb, psm[:, 0:H])
        for h in range(H):
            nc.scalar.activation(e1dmc[:, :, h], dstc[:, :, h], AF.Exp,
                                 bias=negmb[:, h:h + 1])

        # src[i,h] = x_mine @ v_src; transpose to rows; bcast; exp
        srcT = pa.tile([H, MY], F32, tag="srcT")
        for t in range(NIB):
            ps = whp.tile([128, 512], F32, tag="ps")
            nc.tensor.matmul(ps[:, 0:H], xmtf[:, 128 * t:128 * (t + 1)],
                             vall[:, H:2 * H], start=True, stop=True)
            srcblk = pa.tile([128, H], F32, tag="srcblk")
            nc.scalar.copy(srcblk, ps[:, 0:H])
            ps2 = whp.tile([128, 512], F32, tag="ps")
            nc.tensor.transpose(ps2[0:H, 0:128], srcblk, ident[0:128, 0:128])
            nc.scalar.copy(srcT[:, 128 * t:128 * (t + 1)], ps2[0:H, 0:128])
        for h in range(H):
            ps3 = whp.tile([128, 512], F32, tag="ps")
            nc.tensor.matmul(ps3, sel8[:, h, :], srcT,
                             start=True, stop=True)
            nc.scalar.activation(embsb[:, h, :], ps3, AF.Exp, scale=-0.8,
                                 bias=negmb[:, h:h + 1])

        # Wh for all heads (bf16) + ones column. Head 0's lhsT slices are
        # produced first (196ns/chunk copies) so the first attention pass is
        # never production-stalled; heads 1-7 follow and hide under head 0.
        for jc in range(NJC):
            ps = whp.tile([128, NHID], F32, tag="ps")
            nc.tensor.matmul(ps, xtb[:, 128 * jc:128 * (jc + 1)],
                             wallb[:, 0:NHID], start=True, stop=True)
            nc.scalar.copy(whcall[:, jc, 0, 0:NHID], ps)
        for jc in range(NJC):
            ps = whp.tile([128, (H - 1) * NHID], F32, tag="ps")
            nc.tensor.matmul(ps, xtb[:, 128 * jc:128 * (jc + 1)],
                             wallb[:, NHID:], start=True, stop=True)
            nc.scalar.copy(whcall[:, jc, 1:H, 0:NHID],
                           ps.rearrange("p (h d) -> p h d", h=H - 1))
        nc.gpsimd.memset(whcall[:, :, :, NHID:NHID + 1], 1.0)

    # ---------------- attention pass ----------------
    PATTERN2 = [2, 1, 2, 1, 2, 2, 1, 2, 1, 2, 2, 1, 2, 1, 2, 2,
                1, 2, 1, 2, 2, 1, 2, 1, 2, 2, 1, 2, 1, 2, 1, 1]

    def attention(embsb_ap, e02col_of, e1mcol_of, lhsT_of, dt_el=BF16,
                  pattern=None):
        """pt = min(max(embs*e02d_j, e1dm_j), adjBIG); the per-row factor
        e^{-0.8s-m} cancels in the softmax normalization."""
        pattern = pattern or PATTERN
        aggps = agg.tile([NHID + 1, MY], F32, tag="aggps")
        for jc in range(NJC):
            path = pattern[jc % len(pattern)]
            pt = hot.tile([128, MY], dt_el, tag="pt")
            if path == 3:
                a2t = hot.tile([128, MY], dt_el, tag="m08")
                nc.scalar.mul(a2t, embsb_ap, e02col_of(jc))
                nc.vector.scalar_tensor_tensor(pt, a2t, e1mcol_of(jc),
                                               adjtb[:, jc, :],
                                               op0=ALU.max, op1=ALU.mult)
            else:
                m08 = hot.tile([128, MY], dt_el, tag="m08")
                nc.vector.tensor_scalar(m08, embsb_ap, e02col_of(jc),
                                        e1mcol_of(jc),
                                        op0=ALU.mult, op1=ALU.max)
                eng = nc.vector if path == 1 else nc.gpsimd
                eng.tensor_tensor(pt, m08, adjtb[:, jc, :], op=ALU.mult)
            nc.tensor.matmul(aggps, lhsT_of(jc), pt[:],
                             start=(jc == 0), stop=(jc == NJC - 1))
        o = post.tile([NHID + 1, MY], F32, tag="o")
        nc.scalar.copy(o, aggps)
        linv = post.tile([1, MY], F32, tag="linv")
        nc.vector.reciprocal(linv, o[NHID:NHID + 1, :])
        lb = postp.tile([128, 512], F32, tag="pp")
        nc.tensor.matmul(lb[0:NHID, :], ones_row[:, 0:NHID], linv,
                         start=True, stop=True)
        return o, lb

    # ============ PHASE C: layer-1 heads =================================
    for h in range(H):
        o, lb = attention(
            embsb[:, h, :],
            lambda jc, h=h: e02dc[:, jc, h:h + 1],
            lambda jc, h=h: e1dmc[:, jc, h:h + 1],
            lambda jc: whcall[:, jc, h, :])
        scaled = post.tile([NHID, MY], F32, tag="scaled")
        nc.vector.tensor_mul(scaled, o[0:NHID, :], lb[0:NHID, :])
        # ELU(x) = max(x,0) - 1 + exp(min(x,0))
        mm = post.tile([NHID, MY], F32, tag="tmp")
        nc.gpsimd.tensor_scalar(mm, scaled, 0.0, None, op0=ALU.min)
        em = post.tile([NHID, MY], F32, tag="tmp")
        nc.scalar.activation(em, mm, AF.Exp)
        t2 = post.tile([NHID, MY], F32, tag="tmp")
        nc.gpsimd.tensor_scalar(t2, scaled, 0.0, -1.0,
                                op0=ALU.max, op1=ALU.add)
        dst_rows = hcatT[64 * (h % 2):64 * (h % 2) + NHID, h // 2, :]
        nc.vector.tensor_add(dst_rows, em, t2)

    # ============ PHASE D: output attention layer ===============
    p2 = ctx.enter_context(tc.tile_pool(name="p2", bufs=1))

    wh2ps = agg.tile([NHID + 1, MY], F32, tag="aggps")
    for p in range(4):
        nc.tensor.matmul(wh2ps[0:NOUT, :], woutb[:, p, :], hcatT[:, p, :],
                         start=(p == 0), stop=(p == 3))
    wh2Tm = p2.tile([NOUT, MY], F32)
    nc.scalar.copy(wh2Tm, wh2ps[0:NOUT, :])

    ps = postp.tile([128, 512], F32, tag="pp")
    nc.tensor.matmul(ps[0:1, :], a2[:, 1:2], wh2Tm, start=True, stop=True)
    s2r = p2.tile([1, MY], F32)
    nc.scalar.copy(s2r, ps[0:1, :])
    ps = postp.tile([128, 512], F32, tag="pp")
    nc.tensor.matmul(ps, ones_row, s2r, start=True, stop=True)
    srcb2 = p2.tile([128, MY], F32)
    nc.vector.tensor_copy(srcb2, ps)

    # AllGather Wh2.T across cores
    cc_in = dram.tile([NOUT, MY], F32)
    cc_out = dram.tile([N_CORES, NOUT, MY], F32)
    nc.sync.dma_start(out=cc_in, in_=wh2Tm)
    collective("AllGather", ALU.bypass, cc_in, cc_out)
    wh2Tf = p2.tile([NOUT, N], F32)
    for r in range(N_CORES):
        eng = (nc.sync, nc.scalar)[r % 2]
        eng.dma_start(out=wh2Tf[:, MY * r:MY * (r + 1)], in_=cc_out[r])

    dst2ps = postp.tile([128, 512], F32, tag="pp")
    for jc in range(NJC):
        nc.tensor.matmul(dst2ps[:, jc:jc + 1],
                         wh2Tf[:, 128 * jc:128 * (jc + 1)],
                         a2[:, 0:1], start=True, stop=True)
    dst2c = p2.tile([128, NJC], F32)
    nc.scalar.copy(dst2c, dst2ps[:, 0:NJC])
    mx2 = p2.tile([128, 1], F32)
    nc.vector.tensor_reduce(mx2, dst2c, axis=mybir.AxisListType.X, op=ALU.max)
    mx2r = p2.tile([1, 1], F32)
    nc.gpsimd.tensor_reduce(mx2r, mx2, axis=mybir.AxisListType.C, op=ALU.max)
    ps = postp.tile([128, 512], F32, tag="pp")
    nc.tensor.matmul(ps[:, 0:1], negones_row, mx2r, start=True, stop=True)
    negm2 = p2.tile([128, 1], F32)
    nc.vector.tensor_copy(negm2, ps[:, 0:1])
    e1dm2 = p2.tile([128, NJC], F32)
    nc.scalar.activation(e1dm2, dst2c, AF.Exp, bias=negm2[:, 0:1])
    e02d2 = p2.tile([128, NJC], F32)
    nc.scalar.activation(e02d2, dst2c, AF.Exp, scale=0.2)
    embsb2 = p2.tile([128, MY], F32)
    nc.scalar.activation(embsb2, srcb2, AF.Exp, scale=-0.8,
                         bias=negm2[:, 0:1])

    wh2aug = p2.tile([128, NJC, NOUT + 1], F32R)
    for jc in range(NJC):
        ps = postp.tile([128, 512], F32, tag="pp")
        nc.tensor.transpose(ps[:, 0:NOUT], wh2Tf[:, 128 * jc:128 * (jc + 1)],
                            ident[0:NOUT, 0:NOUT])
        nc.scalar.copy(wh2aug[:, jc, 0:NOUT], ps[:, 0:NOUT])
    nc.gpsimd.memset(wh2aug[:, :, NOUT:NOUT + 1].bitcast(F32), 1.0)

    o2, lb2 = attention(
        embsb2,
        lambda jc: e02d2[:, jc:jc + 1],
        lambda jc: e1dm2[:, jc:jc + 1],
        lambda jc: wh2aug[:, jc, :], dt_el=F32R, pattern=PATTERN2)
    out2n = p2.tile([NOUT, MY], F32)
    nc.vector.tensor_mul(out2n, o2[0:NOUT, :], lb2[0:NOUT, :])

    # ============ PHASE E: one-pass batchnorm + tanh + output ============
    # Pack S = sum(x) and S2 = sum(x^2) into one tile -> single AllReduce.
    # var = E[x^2] - mu^2 (values are O(1), numerically fine).
    S = p2.tile([NOUT, 2], F32)
    nc.vector.tensor_reduce(S[:, 0:1], out2n, axis=mybir.AxisListType.X,
                            op=ALU.add)
    sqt = post.tile([NOUT, MY], F32, tag="tmp")
    nc.vector.tensor_mul(sqt, out2n, out2n)
    nc.vector.tensor_reduce(S[:, 1:2], sqt, axis=mybir.AxisListType.X,
                            op=ALU.add)
    bn_in = dram.tile([NOUT, 2], F32)
    bn_out = dram.tile([NOUT, 2], F32)
    nc.sync.dma_start(out=bn_in, in_=S)
    collective("AllReduce", ALU.add, bn_in, bn_out, src_sbuf=S.opt())
    Sg = p2.tile([NOUT, 2], F32)
    nc.sync.dma_start(out=Sg, in_=bn_out)
    mom = p2.tile([NOUT, 2], F32)
    nc.vector.tensor_scalar_mul(mom, Sg, 1.0 / N)   # [mu, E[x^2]]
    mu = mom[:, 0:1]
    negmu = p2.tile([NOUT, 1], F32)
    nc.vector.tensor_scalar_mul(negmu, mu, -1.0)
    var = p2.tile([NOUT, 1], F32)
    nc.vector.tensor_tensor(var, mu, negmu, op=ALU.mult)  # -mu^2
    nc.vector.tensor_add(var, var, mom[:, 1:2])

    epst = p2.tile([NOUT, 1], F32)
    nc.gpsimd.memset(epst, EPS)
    sd = p2.tile([NOUT, 1], F32)
    nc.scalar.activation(sd, var, AF.Sqrt, bias=epst[:, 0:1])
    rstd = p2.tile([NOUT, 1], F32)
    nc.vector.reciprocal(rstd, sd)
    scale = p2.tile([NOUT, 1], F32)
    nc.vector.tensor_mul(scale, rstd, gb[:, 0:1])
    shift = p2.tile([NOUT, 1], F32)
    nc.vector.tensor_mul(shift, negmu, scale)
    nc.vector.tensor_add(shift, shift, gb[:, 1:2])

    finT = p2.tile([NOUT, MY], F32)
    nc.scalar.activation(finT, out2n, AF.Tanh, bias=shift[:, 0:1],
                         scale=scale[:, 0:1])

    fin = p2.tile([128, NIB, NOUT], F32)
    for t in range(NIB):
        ps = postp.tile([128, 512], F32, tag="pp")
        nc.tensor.transpose(ps[:, 0:NOUT], finT[:, 128 * t:128 * (t + 1)],
                            ident[0:NOUT, 0:NOUT])
        nc.scalar.copy(fin[:, t, :], ps[:, 0:NOUT])
    nc.sync.dma_start(out=d["y"].ap().rearrange("(t p) d -> p t d", p=128),
                      in_=fin)


def _prep_inputs(x, adj, W_heads, a_heads, W_out, a_out, gamma, beta):
    """Host-side packing of the small weights + per-core sharding."""
    import ml_dtypes
    x = np.ascontiguousarray(np.asarray(x, dtype=np.float32))
    adj = np.asarray(adj, dtype=np.int32)
    W_heads = np.asarray(W_heads, dtype=np.float32)
    a_heads = np.asarray(a_heads, dtype=np.float32)
    W_out = np.asarray(W_out, dtype=np.float32)
    a_out = np.asarray(a_out, dtype=np.float32)

    xtf = np.ascontiguousarray(x.T)
    wallf = np.ascontiguousarray(
        W_heads.transpose(1, 0, 2).reshape(NIN, H * NHID))
    vall = np.zeros((NIN, 2 * H), np.float32)
    for h in range(H):
        vall[:, h] = W_heads[h] @ a_heads[h, NHID:]        # dst direction
        vall[:, H + h] = W_heads[h] @ a_heads[h, :NHID]    # src direction
    a2 = np.ascontiguousarray(
        np.stack([a_out[NOUT:], a_out[:NOUT]], axis=1).astype(np.float32))
    gb = np.ascontiguousarray(
        np.stack([np.asarray(gamma, np.float32), np.asarray(beta, np.float32)],
                 axis=1))
    eye = np.eye(128, dtype=np.float32)
    sel8 = np.zeros((H, H, 128), np.float32)
    for h in range(H):
        sel8[h, h, :] = 1.0
    woutb = np.ascontiguousarray(
        W_out.reshape(4, 128, NOUT).transpose(1, 0, 2))

    import ml_dtypes as _md
    xtb = np.ascontiguousarray(x.T.astype(_md.bfloat16))
    wallb = wallf.astype(_md.bfloat16)
    shared = {"xtf": xtf, "wallf": wallf, "xtb": xtb, "wallb": wallb,
              "vall": vall,
              "woutb": woutb, "a2": a2, "gb": gb,
              "ident": eye, "sel8": sel8}
    in_maps = []
    adjb = (adj > 0).astype(ml_dtypes.bfloat16)
    for c in range(N_CORES):
        m = dict(shared)
        at = adjb[MY * c:MY * (c + 1)].T            # [4096 j, 512 i]
        m["adjtb"] = np.ascontiguousarray(
            at.reshape(NJC, 128, MY).transpose(1, 0, 2))
        m["xmtf"] = np.ascontiguousarray(xtf[:, MY * c:MY * (c + 1)])
        in_maps.append(m)
    return in_maps


def kernel(x, adj, W_heads, a_heads, W_out, a_out, gamma, beta, **kw):
    if "nc" not in _CACHED:
        _CACHED["nc"] = build_program()
    nc = _CACHED["nc"]
    in_maps = _prep_inputs(x, adj, W_heads, a_heads, W_out, a_out, gamma, beta)
    res = run_bass_kernel_spmd(nc, in_maps, core_ids=list(range(N_CORES)), **kw)
    _CACHED["last_res"] = res
    out = np.concatenate([res.results[c]["y"] for c in range(N_CORES)], axis=0)
    return out


# revision 54
# speedup vs baseline: 1.0057x; 1.0057x over previous
"""GAT (2-layer, 8-head) Trainium2 Bass kernel, 8-core row-parallel SPMD.

Sharding: nodes (rows of x / adj) split across 8 cores, 512 rows each. Each core
computes its rows' attention against all 4096 nodes; small weights replicated.
Attention math runs transposed (source j on partitions, my row i on free dim)
so aggregation attn @ Wh maps onto the PE with no transposition of the big
attention matrix.

Key algebra (per head): with z = src_i + dst_j,
  p = adj * exp(leaky_relu(z)) = adj * e^{0.2 z} * max(e^{0.8 z}, 1)
e^{0.2 s_i} factors out of numerator and denominator of the softmax and
cancels, so per (j-chunk, i) only two elementwise ops remain:
  M08 = max(embs * e02d_j, e1dm_j)      (DVE tensor_scalar, bf16 -> 4x mode)
  PT  = M08 * adjT_j                    (mask multiply; adjT is {0,1} bf16)
  outT[d,i] += [Wh | 1][j,d] * PT[j,i]  (PE matmul bf16, row 64 = denom)
adj is pre-transposed and cast to bf16 {0,1e30} on the host, so no on-device
transposes of the big matrices are needed. The two hot ops per chunk are
distributed over DVE / GPSIMD / ScalarE by a tunable pattern.

Cross-core communication: AllGather of layer-2 Wh2.T (1 MB) and one AllReduce
carrying [sum(x), sum(x^2)] for one-pass batchnorm statistics.
"""

import numpy as np
from contextlib import ExitStack

import concourse.bass as bass
import concourse.bacc as bacc
import concourse.tile as tile
from concourse import mybir
from concourse.bass_utils import run_bass_kernel_spmd

F32 = mybir.dt.float32
F32R = mybir.dt.float32r
BF16 = mybir.dt.bfloat16
I32 = mybir.dt.int32
AF = mybir.ActivationFunctionType
ALU = mybir.AluOpType

N_CORES = 8
N = 4096
NIN = 128
NHID = 64
NOUT = 64
H = 8
MY = N // N_CORES          # 512 rows per core
NJC = N // 128             # 32 j-chunks
NIB = MY // 128            # 4 row blocks per core
ALPHA = 0.2
EPS = 1e-5

CFG = "bf16"

# Per-chunk engine assignment for the two hot elementwise ops, cycled over
# j-chunks. 1: DVE TS + DVE TT(min)   2: DVE TS + Pool TT(min)
# 3: Act mul + DVE STT(max,min) -- strictly worse than 1, unused.
# Balanced so DVE / Pool busy times roughly equalize.
PATTERN = [2, 1, 1, 2, 1, 1, 2, 1, 1, 2, 1, 1, 2, 1, 1, 2,
           1, 1, 2, 1, 1, 2, 1, 1, 2, 1, 1, 2, 1, 2, 1, 1]

_CACHED = {}


def build_program(sim=False, cfg=None):
    nd = 1 if sim else N_CORES
    nc = bacc.Bacc("TRN2", target_bir_lowering=False, debug=False, num_devices=nd)

    d = {}
    d["xtf"] = nc.dram_tensor("xtf", [NIN, N], F32, kind="ExternalInput")
    d["xmtf"] = nc.dram_tensor("xmtf", [NIN, MY], F32, kind="ExternalInput")
    d["adjtb"] = nc.dram_tensor("adjtb", [128, NJC, MY], BF16,
                                kind="ExternalInput")
    d["wallf"] = nc.dram_tensor("wallf", [NIN, H * NHID], F32,
                                kind="ExternalInput")
    d["xtb"] = nc.dram_tensor("xtb", [NIN, N], BF16, kind="ExternalInput")
    d["wallb"] = nc.dram_tensor("wallb", [NIN, H * NHID], BF16,
                                kind="ExternalInput")
    d["vall"] = nc.dram_tensor("vall", [NIN, 2 * H], F32, kind="ExternalInput")
    d["sel8"] = nc.dram_tensor("sel8", [H, H, 128], F32, kind="ExternalInput")
    d["woutb"] = nc.dram_tensor("woutb", [128, 4, NOUT], F32,
                                kind="ExternalInput")
    d["a2"] = nc.dram_tensor("a2", [NOUT, 2], F32, kind="ExternalInput")
    d["gb"] = nc.dram_tensor("gb", [NOUT, 2], F32, kind="ExternalInput")
    d["ident"] = nc.dram_tensor("ident", [128, 128], F32, kind="ExternalInput")
    d["y"] = nc.dram_tensor("y", [MY, NOUT], F32, kind="ExternalOutput")

    with tile.TileContext(nc) as tc:
        with ExitStack() as ctx:
            _build_body(nc, tc, ctx, d, sim=sim)
    nc.compile()
    return nc


def _build_body(nc, tc, ctx, d, sim):
    def collective(kind, op, in_tile, out_tile, src_sbuf=None):
        if sim:
            if kind == "AllGather":
                nc.sync.dma_start(out=out_tile[0], in_=in_tile.opt())
            elif src_sbuf is not None:
                nc.sync.dma_start(out=out_tile.opt(), in_=src_sbuf)
            else:
                nc.sync.dma_start(out=out_tile.opt(), in_=in_tile.opt())
        else:
            nc.gpsimd.collective_compute(
                kind, op, replica_groups=[list(range(N_CORES))],
                ins=[in_tile.opt()], outs=[out_tile.opt()])

    consts = ctx.enter_context(tc.tile_pool(name="consts", bufs=1))
    persist = ctx.enter_context(tc.tile_pool(name="persist", bufs=1))
    dram = ctx.enter_context(tc.tile_pool(name="dram", bufs=1, space="DRAM"))
    hot = ctx.enter_context(tc.tile_pool(name="hot", bufs=10))
    wh2pp = ctx.enter_context(tc.tile_pool(name="wh2p", bufs=1, space="PSUM"))
    agg = ctx.enter_context(tc.tile_pool(name="agg", bufs=2, space="PSUM"))
    post = ctx.enter_context(tc.tile_pool(name="post", bufs=2))
    postp = ctx.enter_context(tc.tile_pool(name="postp", bufs=2, space="PSUM"))
    ctx_a = ExitStack()

    # ---------------- inputs to SBUF ----------------
    # Critical phase-A inputs dispatch first, spread over the two HWDGE
    # queues (SP / Act) to limit serial descriptor-generation delay.
    adjtb = persist.tile([128, NJC, MY], BF16)   # {0,1}; adj[i, 128*jc+jp]

    def adjtb_dma(q, eng):
        eng.dma_start(out=adjtb[:, 4 * q:4 * (q + 1), :],
                      in_=d["adjtb"].ap()[:, 4 * q:4 * (q + 1), :])

    inp = ctx_a.enter_context(tc.tile_pool(name="inp", bufs=1))
    xtf = inp.tile([128, N], F32)
    nc.sync.dma_start(out=xtf, in_=d["xtf"].ap())
    adjtb_dma(0, nc.scalar)
    vall = inp.tile([128, 2 * H], F32)
    nc.scalar.dma_start(out=vall, in_=d["vall"].ap())
    xmtf = inp.tile([128, MY], F32)
    nc.scalar.dma_start(out=xmtf, in_=d["xmtf"].ap())
    adjtb_dma(1, nc.sync)
    sel8 = inp.tile([H, H, 128], F32)
    nc.scalar.dma_start(out=sel8, in_=d["sel8"].ap())
    wallb = inp.tile([128, H * NHID], BF16)
    nc.scalar.dma_start(out=wallb, in_=d["wallb"].ap())
    xtb = inp.tile([128, N], BF16)
    nc.sync.dma_start(out=xtb, in_=d["xtb"].ap())
    ident = consts.tile([128, 128], F32)
    nc.sync.dma_start(out=ident, in_=d["ident"].ap())
    for q, eng in ((2, nc.scalar), (3, nc.sync), (4, nc.scalar),
                   (5, nc.scalar), (6, nc.sync), (7, nc.sync)):
        adjtb_dma(q, eng)

    ones_row = consts.tile([1, 128], F32)
    nc.gpsimd.memset(ones_row, 1.0)
    negones_row = consts.tile([1, 128], F32)
    nc.gpsimd.memset(negones_row, -1.0)

    woutb = consts.tile([128, 4, NOUT], F32)
    nc.sync.dma_start(out=woutb, in_=d["woutb"].ap())
    a2 = consts.tile([NOUT, 2], F32)
    nc.scalar.dma_start(out=a2, in_=d["a2"].ap())
    gb = consts.tile([NOUT, 2], F32)
    nc.sync.dma_start(out=gb, in_=d["gb"].ap())

        # ---------------- persistent intermediates ----------------
    dstc = persist.tile([128, NJC, H], F32)     # per-head dst[128*jc+jp]
    e1dmc = persist.tile([128, NJC, H], F32)    # exp(dst - m_h)
    e02dc = persist.tile([128, NJC, H], F32)    # exp(0.2*dst)
    embsb = persist.tile([128, H, MY], BF16)    # exp(-0.8*src - m_h) bcast
    negmb = persist.tile([128, H], F32)         # -m_h (per-head dst max)
    whcall = persist.tile([128, NJC, H, NHID + 1], BF16)  # [Wh | 1] lhsT
    hcatT = persist.tile([128, NIB, MY], F32)   # layer-1 output (transposed)


    # ============ PHASE A: x-side precompute ==============
    with ctx_a as actx:
        pa = actx.enter_context(tc.tile_pool(name="pa", bufs=4))
        whp = actx.enter_context(tc.tile_pool(name="whp", bufs=3, space="PSUM"))

        # dst[j,h] = x[j,:] @ (W_h @ a_dst_h) with host-precomputed v
        dstps = whp.tile([128, NJC, H], F32, tag="ps")
        for jc in range(NJC):
            nc.tensor.matmul(dstps[:, jc, :], xtf[:, 128 * jc:128 * (jc + 1)],
                             vall[:, 0:H], start=True, stop=True)
        nc.vector.tensor_copy(dstc, dstps)
        nc.scalar.activation(e02dc, dstc, AF.Exp, scale=0.2)
        # per-head m_h = max_j dst_h[j]; negmb[:, h] = -m_h broadcast
        mxp = pa.tile([128, H], F32, tag="mxp")
        for h in range(H):
            nc.vector.tensor_reduce(mxp[:, h:h + 1], dstc[:, :, h],
                                    axis=mybir.AxisListType.X, op=ALU.max)
        mxrow = pa.tile([1, H], F32, tag="mxrow")
        nc.gpsimd.tensor_reduce(mxrow, mxp, axis=mybir.AxisListType.C,
                                op=ALU.max)
        psm = whp.tile([128, 512], F32, tag="ps")
        nc.tensor.matmul(psm[:, 0:H], negones_row, mxrow,
                         start=True, stop=True)
        nc.vector.tensor_copy(negmb, psm[:, 0:H])
        for h in range(H):
            nc.scalar.activation(e1dmc[:, :, h], dstc[:, :, h], AF.Exp,
                                 bias=negmb[:, h:h + 1])

        # src[i,h] = x_mine @ v_src; transpose to rows; bcast; exp
        srcT = pa.tile([H, MY], F32, tag="srcT")
        for t in range(NIB):
            ps = whp.tile([128, 512], F32, tag="ps")
            nc.tensor.matmul(ps[:, 0:H], xmtf[:, 128 * t:128 * (t + 1)],
                             vall[:, H:2 * H], start=True, stop=True)
            srcblk = pa.tile([128, H], F32, tag="srcblk")
            nc.vector.tensor_copy(srcblk, ps[:, 0:H])
            ps2 = whp.tile([128, 512], F32, tag="ps")
            nc.tensor.transpose(ps2[0:H, 0:128], srcblk, ident[0:128, 0:128])
            nc.vector.tensor_copy(srcT[:, 128 * t:128 * (t + 1)],
                                  ps2[0:H, 0:128])
        for h in range(H):
            ps3 = whp.tile([128, 512], F32, tag="ps")
            nc.tensor.matmul(ps3, sel8[:, h, :], srcT,
                             start=True, stop=True)
            nc.scalar.activation(embsb[:, h, :], ps3, AF.Exp, scale=-0.8,
                                 bias=negmb[:, h:h + 1])

        # Wh for all heads (bf16) + ones column. Head 0's lhsT slices are
        # produced first (196ns/chunk copies) so the first attention pass is
        # never production-stalled; heads 1-7 follow and hide under head 0.
        for jc in range(NJC):
            ps = whp.tile([128, NHID], F32, tag="ps")
            nc.tensor.matmul(ps, xtb[:, 128 * jc:128 * (jc + 1)],
                             wallb[:, 0:NHID], start=True, stop=True)
            nc.scalar.copy(whcall[:, jc, 0, 0:NHID], ps)
        for jc in range(NJC):
            ps = whp.tile([128, (H - 1) * NHID], F32, tag="ps")
            nc.tensor.matmul(ps, xtb[:, 128 * jc:128 * (jc + 1)],
                             wallb[:, NHID:], start=True, stop=True)
            nc.scalar.copy(whcall[:, jc, 1:H, 0:NHID],
                           ps.rearrange("p (h d) -> p h d", h=H - 1))
        nc.gpsimd.memset(whcall[:, :, :, NHID:NHID + 1], 1.0)

    # ---------------- attention pass ----------------
    PATTERN2 = [2, 1, 2, 1, 2, 2, 1, 2, 1, 2, 2, 1, 2, 1, 2, 2,
                1, 2, 1, 2, 2, 1, 2, 1, 2, 2, 1, 2, 1, 2, 1, 1]

    def attention(embsb_ap, e02col_of, e1mcol_of, lhsT_of, dt_el=BF16,
                  pattern=None):
        """pt = min(max(embs*e02d_j, e1dm_j), adjBIG); the per-row factor
        e^{-0.8s-m} cancels in the softmax normalization."""
        pattern = pattern or PATTERN
        aggps = agg.tile([NHID + 1, MY], F32, tag="aggps")
        for jc in range(NJC):
            path = pattern[jc % len(pattern)]
            pt = hot.tile([128, MY], dt_el, tag="pt")
            if path == 3:
                a2t = hot.tile([128, MY], dt_el, tag="m08")
                nc.scalar.mul(a2t, embsb_ap, e02col_of(jc))
                nc.vector.scalar_tensor_tensor(pt, a2t, e1mcol_of(jc),
                                               adjtb[:, jc, :],
                                               op0=ALU.max, op1=ALU.mult)
            else:
                m08 = hot.tile([128, MY], dt_el, tag="m08")
                nc.vector.tensor_scalar(m08, embsb_ap, e02col_of(jc),
                                        e1mcol_of(jc),
                                        op0=ALU.mult, op1=ALU.max)
                eng = nc.vector if path == 1 else nc.gpsimd
                eng.tensor_tensor(pt, m08, adjtb[:, jc, :], op=ALU.mult)
            nc.tensor.matmul(aggps, lhsT_of(jc), pt[:],
                             start=(jc == 0), stop=(jc == NJC - 1))
        o = post.tile([NHID + 1, MY], F32, tag="o")
        nc.scalar.copy(o, aggps)
        linv = post.tile([1, MY], F32, tag="linv")
        nc.vector.reciprocal(linv, o[NHID:NHID + 1, :])
        lb = postp.tile([128, 512], F32, tag="pp")
        nc.tensor.matmul(lb[0:NHID, :], ones_row[:, 0:NHID], linv,
                         start=True, stop=True)
        return o, lb

    # ============ PHASE C: layer-1 heads =================================
    wh2ps = wh2pp.tile([NHID + 1, MY], F32, tag="wh2ps")
    for h in range(H):
        o, lb = attention(
            embsb[:, h, :],
            lambda jc, h=h: e02dc[:, jc, h:h + 1],
            lambda jc, h=h: e1dmc[:, jc, h:h + 1],
            lambda jc: whcall[:, jc, h, :])
        scaled = post.tile([NHID, MY], F32, tag="scaled")
        nc.vector.tensor_mul(scaled, o[0:NHID, :], lb[0:NHID, :])
        # ELU(x) = max(x,0) - 1 + exp(min(x,0))
        mm = post.tile([NHID, MY], F32, tag="tmp")
        nc.gpsimd.tensor_scalar(mm, scaled, 0.0, None, op0=ALU.min)
        em = post.tile([NHID, MY], F32, tag="tmp")
        nc.scalar.activation(em, mm, AF.Exp)
        t2 = post.tile([NHID, MY], F32, tag="tmp")
        nc.gpsimd.tensor_scalar(t2, scaled, 0.0, -1.0,
                                op0=ALU.max, op1=ALU.add)
        dst_rows = hcatT[64 * (h % 2):64 * (h % 2) + NHID, h // 2, :]
        nc.vector.tensor_add(dst_rows, em, t2)
        if h % 2 == 1:
            nc.tensor.matmul(wh2ps[0:NOUT, :], woutb[:, h // 2, :],
                             hcatT[:, h // 2, :], start=(h == 1), stop=(h == 7))
        if h == 0:
            # pull the Sqrt/Tanh activation-table loads off the serial tail
            epst = consts.tile([NOUT, 1], F32)
            nc.gpsimd.memset(epst, EPS)
            warm = post.tile([1, 1], F32, tag="warm")
            nc.scalar.activation(warm, epst[0:1, 0:1], AF.Sqrt)
            warm2 = post.tile([1, 1], F32, tag="warm")
            nc.scalar.activation(warm2, epst[0:1, 0:1], AF.Tanh)

    # ============ PHASE D: output attention layer ===============
    p2 = ctx.enter_context(tc.tile_pool(name="p2", bufs=1))

    wh2Tm = p2.tile([NOUT, MY], F32)
    nc.scalar.copy(wh2Tm, wh2ps[0:NOUT, :])

    ps = postp.tile([128, 512], F32, tag="pp")
    nc.tensor.matmul(ps[0:1, :], a2[:, 1:2], wh2Tm, start=True, stop=True)
    s2r = p2.tile([1, MY], F32)
    nc.scalar.copy(s2r, ps[0:1, :])
    ps = postp.tile([128, 512], F32, tag="pp")
    nc.tensor.matmul(ps, ones_row, s2r, start=True, stop=True)
    srcb2 = p2.tile([128, MY], F32)
    nc.vector.tensor_copy(srcb2, ps)

    # AllGather Wh2.T across cores
    cc_in = dram.tile([NOUT, MY], F32)
    cc_out = dram.tile([N_CORES, NOUT, MY], F32)
    nc.sync.dma_start(out=cc_in, in_=wh2Tm)
    collective("AllGather", ALU.bypass, cc_in, cc_out)
    wh2Tf = p2.tile([NOUT, N], F32)
    for r in range(N_CORES):
        eng = (nc.sync, nc.scalar)[r % 2]
        eng.dma_start(out=wh2Tf[:, MY * r:MY * (r + 1)], in_=cc_out[r])

    dst2ps = postp.tile([128, 512], F32, tag="pp")
    for jc in range(NJC):
        nc.tensor.matmul(dst2ps[:, jc:jc + 1],
                         wh2Tf[:, 128 * jc:128 * (jc + 1)],
                         a2[:, 0:1], start=True, stop=True)
    dst2c = p2.tile([128, NJC], F32)
    nc.scalar.copy(dst2c, dst2ps[:, 0:NJC])
    mx2 = p2.tile([128, 1], F32)
    nc.vector.tensor_reduce(mx2, dst2c, axis=mybir.AxisListType.X, op=ALU.max)
    mx2r = p2.tile([1, 1], F32)
    nc.gpsimd.tensor_reduce(mx2r, mx2, axis=mybir.AxisListType.C, op=ALU.max)
    ps = postp.tile([128, 512], F32, tag="pp")
    nc.tensor.matmul(ps[:, 0:1], negones_row, mx2r, start=True, stop=True)
    negm2 = p2.tile([128, 1], F32)
    nc.vector.tensor_copy(negm2, ps[:, 0:1])
    e1dm2 = p2.tile([128, NJC], F32)
    nc.scalar.activation(e1dm2, dst2c, AF.Exp, bias=negm2[:, 0:1])
    e02d2 = p2.tile([128, NJC], F32)
    nc.scalar.activation(e02d2, dst2c, AF.Exp, scale=0.2)
    embsb2 = p2.tile([128, MY], F32)
    nc.scalar.activation(embsb2, srcb2, AF.Exp, scale=-0.8,
                         bias=negm2[:, 0:1])

    wh2aug = p2.tile([128, NJC, NOUT + 1], F32R)
    for jc in range(NJC):
        ps = postp.tile([128, 512], F32, tag="pp")
        nc.tensor.transpose(ps[:, 0:NOUT], wh2Tf[:, 128 * jc:128 * (jc + 1)],
                            ident[0:NOUT, 0:NOUT])
        nc.scalar.copy(wh2aug[:, jc, 0:NOUT], ps[:, 0:NOUT])
    nc.gpsimd.memset(wh2aug[:, :, NOUT:NOUT + 1].bitcast(F32), 1.0)

    o2, lb2 = attention(
        embsb2,
        lambda jc: e02d2[:, jc:jc + 1],
        lambda jc: e1dm2[:, jc:jc + 1],
        lambda jc: wh2aug[:, jc, :], dt_el=F32R, pattern=PATTERN2)
    out2n = p2.tile([NOUT, MY], F32)
    nc.vector.tensor_mul(out2n, o2[0:NOUT, :], lb2[0:NOUT, :])

    # ============ PHASE E: one-pass batchnorm + tanh + output ============
    # Pack S = sum(x) and S2 = sum(x^2) into one tile -> single AllReduce.
    # var = E[x^2] - mu^2 (values are O(1), numerically fine).
    S = p2.tile([NOUT, 2], F32)
    nc.vector.tensor_reduce(S[:, 0:1], out2n, axis=mybir.AxisListType.X,
                            op=ALU.add)
    sqt = post.tile([NOUT, MY], F32, tag="tmp")
    nc.vector.tensor_mul(sqt, out2n, out2n)
    nc.vector.tensor_reduce(S[:, 1:2], sqt, axis=mybir.AxisListType.X,
                            op=ALU.add)
    bn_in = dram.tile([NOUT, 2], F32)
    bn_out = dram.tile([NOUT, 2], F32)
    nc.sync.dma_start(out=bn_in, in_=S)
    collective("AllReduce", ALU.add, bn_in, bn_out, src_sbuf=S.opt())
    Sg = p2.tile([NOUT, 2], F32)
    nc.sync.dma_start(out=Sg, in_=bn_out)
    mom = p2.tile([NOUT, 2], F32)
    nc.vector.tensor_scalar_mul(mom, Sg, 1.0 / N)   # [mu, E[x^2]]
    mu = mom[:, 0:1]
    musq = p2.tile([NOUT, 1], F32)
    nc.vector.tensor_mul(musq, mu, mu)
    var = p2.tile([NOUT, 1], F32)
    nc.vector.tensor_tensor(var, mom[:, 1:2], musq, op=ALU.subtract)
    sd = p2.tile([NOUT, 1], F32)
    nc.scalar.activation(sd, var, AF.Sqrt, bias=epst[:, 0:1])
    rstd = p2.tile([NOUT, 1], F32)
    nc.vector.reciprocal(rstd, sd)
    scale = p2.tile([NOUT, 1], F32)
    nc.vector.tensor_mul(scale, rstd, gb[:, 0:1])
    ms = p2.tile([NOUT, 1], F32)
    nc.vector.tensor_mul(ms, mu, scale)
    shift = p2.tile([NOUT, 1], F32)
    nc.vector.tensor_tensor(shift, gb[:, 1:2], ms, op=ALU.subtract)

    finT = p2.tile([NOUT, MY], F32)
    nc.scalar.activation(finT, out2n, AF.Tanh, bias=shift[:, 0:1],
                         scale=scale[:, 0:1])

    fin = p2.tile([128, NIB, NOUT], F32)
    for t in range(NIB):
        ps = postp.tile([128, 512], F32, tag="pp")
        nc.tensor.transpose(ps[:, 0:NOUT], finT[:, 128 * t:128 * (t + 1)],
                            ident[0:NOUT, 0:NOUT])
        nc.scalar.copy(fin[:, t, :], ps[:, 0:NOUT])
    nc.sync.dma_start(out=d["y"].ap().rearrange("(t p) d -> p t d", p=128),
                      in_=fin)


def _prep_inputs(x, adj, W_heads, a_heads, W_out, a_out, gamma, beta):
    """Host-side packing of the small weights + per-core sharding."""
    import ml_dtypes
    x = np.ascontiguousarray(np.asarray(x, dtype=np.float32))
    adj = np.asarray(adj, dtype=np.int32)
    W_heads = np.asarray(W_heads, dtype=np.float32)
    a_heads = np.asarray(a_heads, dtype=np.float32)
    W_out = np.asarray(W_out, dtype=np.float32)
    a_out = np.asarray(a_out, dtype=np.float32)

    xtf = np.ascontiguousarray(x.T)
    wallf = np.ascontiguousarray(
        W_heads.transpose(1, 0, 2).reshape(NIN, H * NHID))
    vall = np.zeros((NIN, 2 * H), np.float32)
    for h in range(H):
        vall[:, h] = W_heads[h] @ a_heads[h, NHID:]        # dst direction
        vall[:, H + h] = W_heads[h] @ a_heads[h, :NHID]    # src direction
    a2 = np.ascontiguousarray(
        np.stack([a_out[NOUT:], a_out[:NOUT]], axis=1).astype(np.float32))
    gb = np.ascontiguousarray(
        np.stack([np.asarray(gamma, np.float32), np.asarray(beta, np.float32)],
                 axis=1))
    eye = np.eye(128, dtype=np.float32)
    sel8 = np.zeros((H, H, 128), np.float32)
    for h in range(H):
        sel8[h, h, :] = 1.0
    woutb = np.ascontiguousarray(
        W_out.reshape(4, 128, NOUT).transpose(1, 0, 2))

    import ml_dtypes as _md
    xtb = np.ascontiguousarray(x.T.astype(_md.bfloat16))
    wallb = wallf.astype(_md.bfloat16)
    shared = {"xtf": xtf, "wallf": wallf, "xtb": xtb, "wallb": wallb,
              "vall": vall,
              "woutb": woutb, "a2": a2, "gb": gb,
              "ident": eye, "sel8": sel8}
    in_maps = []
    adjb = (adj > 0).astype(ml_dtypes.bfloat16)
    for c in range(N_CORES):
        m = dict(shared)
        at = adjb[MY * c:MY * (c + 1)].T            # [4096 j, 512 i]
        m["adjtb"] = np.ascontiguousarray(
            at.reshape(NJC, 128, MY).transpose(1, 0, 2))
        m["xmtf"] = np.ascontiguousarray(xtf[:, MY * c:MY * (c + 1)])
        in_maps.append(m)
    return in_maps


def kernel(x, adj, W_heads, a_heads, W_out, a_out, gamma, beta, **kw):
    if "nc" not in _CACHED:
        _CACHED["nc"] = build_program()
    nc = _CACHED["nc"]
    in_maps = _prep_inputs(x, adj, W_heads, a_heads, W_out, a_out, gamma, beta)
    res = run_bass_kernel_spmd(nc, in_maps, core_ids=list(range(N_CORES)), **kw)
    _CACHED["last_res"] = res
    out = np.concatenate([res.results[c]["y"] for c in range(N_CORES)], axis=0)
    return out
